# revision 3
# baseline (speedup 1.0000x reference)
"""Trainium2 Bass kernel for nn_AuxiliaryConditionerBlock.

Math (reference):
  pos_feat = [cos(2*pi*pos), sin(2*pi*pos)]                    [S, 6]
  val      = (nodes @ Wv.T + bv).reshape(S, 24, 128)
  k/q      = per-source projections -> concat over heads       [S, 24, 32]
  logits[i,j,h] = sum_c k[i,h,c] * q[j,h,c]  (last 8 heads squared)
  att      = softmax_j(logits);  out[i] = sum_{j,h} att[i,j,h] * val[j,h]

Kernel strategy (8 NeuronCores, sequence-parallel over output rows i):
  - each core owns 256 output rows (the k-side rows); q/values replicated
  - column-major scores per head: scores_T[j, i] = qT_chunk.T @ kT_loc
    (K=32 per head, 4 heads row-tiled into the 128x128 PE array, one
    PSUM bank per head -- concurrent same-bank PE writes are fatal)
  - exp on ScalarE (PSUM->SBUF, bf16 att out, FD=2048 tiles)
  - squared heads: ACT Square in place, then DVE add of replicated
    per-row offsets -C_i (host-computed row maxima; offsets cancel in
    softmax exactly, they only keep exp in fp32 range)
  - attention*values factored: ctx_h = att_h.T @ [nodes | 1] gives the
    24x129-dim context + softmax denominators in one accumulation;
    val projection applied after: out = sum_h (ctx_h/denom_h) @ WvT_h
  - ctx normalized per-row (DVE), transposed on PE, and contracted with
    w_values.T (bf16) accumulating in a single PSUM bank.
"""

import os
import sys
import types

import numpy as np
import ml_dtypes

S = 2048
D = 128
NCORES = 8
SL = S // NCORES          # 256 local rows per core
H3 = 24                   # total heads
NHG = 6                   # head-groups of 4
NCHUNK = 16               # j-chunks of 128

bf16 = ml_dtypes.bfloat16

_PROGRAM_CACHE = {}


def _install_env_patches():
    """Inject the missing antenv.axon_hooks module (NTFF profiling) and
    patch Tile's exit drain, whose aggregated sem waits exceed the walrus
    per-instruction sync-wait limit."""
    import antenv
    if "antenv.axon_hooks" not in sys.modules:
        mod = types.ModuleType("antenv.axon_hooks")
        _h = [None]
        mod.set_axon_ntff_profile_hook = lambda h: _h.__setitem__(0, h)
        mod.get_axon_ntff_profile_hook = lambda: _h[0]
        sys.modules["antenv.axon_hooks"] = mod
        antenv.axon_hooks = mod
        try:
            from trn_agent_boot.trn_boot import _ntff_profile_via_ctypes
            mod.set_axon_ntff_profile_hook(
                _ntff_profile_via_ctypes('/opt/axon/libaxon_pjrt.so'))
        except Exception:
            pass

    import concourse.mybir as mybir
    import concourse.tile as tile
    from concourse.vector_clock import ScopedClock

    def _patched_drain_and_barrier(self, tick_clock, wait_clock):
        drain_inst = self.nc.sync.drain()
        wait_clock.add_sem_waits(
            drain_inst.ins, ScopedClock({None: tick_clock.global_clock}))
        si = drain_inst.ins.sync_info
        if si is not None and si.on_wait and len(si.on_wait) > 1:
            waits = list(si.on_wait)
            drain_inst.ins.sync_info = mybir.SyncInfo(
                on_wait=waits[:1], on_update=list(si.on_update))
            for w in waits[1:]:
                nop = self.nc.sync.nop()
                nop.ins.sync_info = mybir.SyncInfo(on_wait=[w], on_update=[])
        self.nc.all_engine_barrier()
        assert self.sems is not None
        popped = self.nc._tile_sem_poison_stack.pop()
        assert popped is self._sem_poison
        self.nc.clear_and_free_semaphores(list(self.sems.allocated().values()))
        self.nc.all_engine_barrier()

    tile.TileContext._drain_and_barrier = _patched_drain_and_barrier


_WS_CTR = [0]


def _fix_sync_waits(nc, maxw=1):
    """walrus codegen rejects instructions with too many sync waits;
    split excess waits onto same-engine NOPs placed just before."""
    import concourse.mybir as mybir
    for fn in nc.m.functions:
        for bb in fn.blocks:
            out = []
            changed = False
            for inst in bb.instructions:
                si = inst.sync_info
                if si is not None and si.on_wait and len(si.on_wait) > maxw:
                    waits = list(si.on_wait)
                    extra, keep = waits[:-maxw], waits[-maxw:]
                    for i in range(0, len(extra), maxw):
                        _WS_CTR[0] += 1
                        nop = mybir.InstNoOp(
                            name=f"zz_waitsplit_{_WS_CTR[0]}", ins=[], outs=[],
                            engine=inst.engine)
                        nop.sync_info = mybir.SyncInfo(
                            on_wait=extra[i:i + maxw], on_update=[])
                        out.append(nop)
                    inst.sync_info = mybir.SyncInfo(
                        on_wait=keep, on_update=list(si.on_update))
                    changed = True
                out.append(inst)
            if changed:
                bb.instructions = out


def build_program():
    import concourse.bass as bass
    import concourse.mybir as mybir
    import concourse.tile as tile
    from concourse.masks import make_identity

    f32 = mybir.dt.float32
    bfl = mybir.dt.bfloat16
    AF = mybir.ActivationFunctionType

    nc = bass.Bass("TRN2", target_bir_lowering=False, debug=False,
                   num_devices=NCORES)

    # ---- DRAM I/O -------------------------------------------------------
    nodesT = nc.dram_tensor("nodesT", [D, S], f32, kind="ExternalInput")
    nodesT_loc = nc.dram_tensor("nodesT_loc", [D, SL], f32, kind="ExternalInput")
    auxT = nc.dram_tensor("auxT", [11, S], f32, kind="ExternalInput")
    auxT_loc = nc.dram_tensor("auxT_loc", [11, SL], f32, kind="ExternalInput")
    n1d = nc.dram_tensor("n1", [D, NCHUNK * 129], bfl, kind="ExternalInput")
    WnT = nc.dram_tensor("WnT", [D, 512], f32, kind="ExternalInput")
    bnT = nc.dram_tensor("bnT", [D, 4], f32, kind="ExternalInput")
    WauxT = nc.dram_tensor("WauxT", [11, 1024], f32, kind="ExternalInput")
    WvT = nc.dram_tensor("WvT", [D, H3 * D], bfl, kind="ExternalInput")
    bvs = nc.dram_tensor("bvs", [1, D], f32, kind="ExternalInput")
    Crep = nc.dram_tensor("Crep", [2, 2048], f32, kind="ExternalInput")
    outd = nc.dram_tensor("out", [SL, D], f32, kind="ExternalOutput")

    with tile.TileContext(nc) as tc:
        from contextlib import ExitStack
        with ExitStack() as ctx:
            sb = ctx.enter_context(tc.tile_pool(name="sb", bufs=1))
            attp = ctx.enter_context(tc.tile_pool(name="attp", bufs=3))
            evacp = ctx.enter_context(tc.tile_pool(name="evacp", bufs=4))
            scoresp = ctx.enter_context(
                tc.tile_pool(name="scoresp", bufs=1, space="PSUM"))

            # ---- SBUF loads ------------------------------------------
            wn = sb.tile([D, 512], f32)
            nc.sync.dma_start(out=wn[:], in_=WnT[:])
            ntl = sb.tile([D, SL], f32)
            nc.sync.dma_start(out=ntl[:], in_=nodesT_loc[:])
            nt = sb.tile([D, S], f32)
            nc.sync.dma_start(out=nt[:], in_=nodesT[:])
            axl = sb.tile([11, SL], f32)
            nc.sync.dma_start(out=axl[:], in_=auxT_loc[:])
            ax = sb.tile([11, S], f32)
            nc.sync.dma_start(out=ax[:], in_=auxT[:])
            bn = sb.tile([D, 4], f32)
            nc.sync.dma_start(out=bn[:], in_=bnT[:])
            wx = sb.tile([11, 1024], f32)
            nc.sync.dma_start(out=wx[:], in_=WauxT[:])
            n1 = sb.tile([D, NCHUNK, 129], bfl)
            nc.sync.dma_start(
                out=n1[:].rearrange("p a b -> p (a b)"), in_=n1d[:])
            wv = sb.tile([D, H3 * D], bfl)
            nc.sync.dma_start(out=wv[:], in_=WvT[:])
            crt = sb.tile([D, 2, 2048], f32)
            for g in range(2):
                nc.gpsimd.dma_start(
                    out=crt[:, g, :], in_=Crep[g:g + 1, :].to_broadcast((D, 2048)))
            bvr = sb.tile([D, D], f32)
            nc.gpsimd.dma_start(out=bvr[:], in_=bvs[0:1, :].to_broadcast((D, D)))
            ident = sb.tile([D, D], bfl)
            make_identity(nc, ident[:])

            # ---- projections -> qT/kT per head-group (bf16) ----------
            qT = []
            kT = []
            for hg in range(NHG):
                dt_hg = f32 if hg >= 4 else bfl
                qT.append(sb.tile([D, S], dt_hg, name=f"qT{hg}"))
                kT.append(sb.tile([D, SL], dt_hg, name=f"kT{hg}"))

            # nodes-family: chunks 0,1 = k(hg0,hg1); 2,3 = q(hg0,hg1)
            projp = tc.alloc_tile_pool(name="projp", bufs=2, space="PSUM")
            for m in range(4):
                lhs = wn[:, m * 128:(m + 1) * 128]
                is_q = m >= 2
                hg = m - 2 if is_q else m
                if not is_q:
                    ps = projp.tile([D, 512], f32, tag="proj", name=f"pn{m}")
                    ps = ps[:, 0:SL]
                    nc.tensor.matmul(ps[:], lhs, ntl[:], start=True, stop=True)
                    nc.vector.tensor_scalar_add(kT[hg][:], ps[:], bn[:, m:m + 1])
                else:
                    for b in range(4):
                        ps = projp.tile([D, 512], f32, tag="proj",
                                        name=f"pn{m}_{b}")
                        nc.tensor.matmul(
                            ps[:], lhs, nt[:, b * 512:(b + 1) * 512],
                            start=True, stop=True)
                        nc.vector.tensor_scalar_add(
                            qT[hg][:, b * 512:(b + 1) * 512], ps[:],
                            bn[:, m:m + 1])
            # aux-family: chunks 0..3 = k(hg2..hg5); 4..7 = q(hg2..hg5)
            for a in range(8):
                lhs = wx[:, a * 128:(a + 1) * 128]
                is_q = a >= 4
                hg = 2 + (a - 4 if is_q else a)
                if not is_q:
                    ps = projp.tile([D, 512], f32, tag="proj", name=f"pa{a}")
                    ps = ps[:, 0:SL]
                    nc.tensor.matmul(ps[:], lhs, axl[:], start=True, stop=True)
                    nc.scalar.copy(kT[hg][:], ps[:])
                else:
                    for b in range(4):
                        ps = projp.tile([D, 512], f32, tag="proj",
                                        name=f"pa{a}_{b}")
                        nc.tensor.matmul(
                            ps[:], lhs, ax[:, b * 512:(b + 1) * 512],
                            start=True, stop=True)
                        nc.scalar.copy(qT[hg][:, b * 512:(b + 1) * 512], ps[:])

            projp.release()

            # ---- main loop -------------------------------------------
            ctxp = ctx.enter_context(
                tc.tile_pool(name="ctxp", bufs=3, space="PSUM"))
            outp_pool = ctx.enter_context(
                tc.tile_pool(name="outp", bufs=1, space="PSUM"))
            out_ps = outp_pool.tile([D, 2, D], f32)

            for hg in range(NHG):
                rot = hg >= 4
                ctx_tiles = [ctxp.tile([D, 3, 130], f32, tag="ctx",
                                       name=f"ctx{hg}_{i}") for i in range(3)]

                def ctx_slot(h, it):
                    s = h * 2 + it
                    return ctx_tiles[s // 3][:, s % 3, 0:129]

                for ct in range(8):
                    sc = scoresp.tile([D, 4, 2, 256], f32, tag="scores",
                                      name=f"sc{hg}_{ct}")
                    for cc in range(2):
                        c = 2 * ct + cc
                        for h in range(4):
                            nc.tensor.matmul(
                                sc[:, h, cc, :],
                                qT[hg][32 * h:32 * h + 32, c * 128:(c + 1) * 128],
                                kT[hg][32 * h:32 * h + 32, :],
                                start=True, stop=True,
                                tile_position=(32 * h, 0))
                    flat = sc[:].rearrange("p a b c -> p (a b c)")
                    if rot:
                        nc.scalar.activation(flat, flat, AF.Square)
                        nc.vector.tensor_add(flat, flat, crt[:, hg - 4, :])
                    att = attp.tile([D, 4, 2, 256], bfl, tag="att",
                                    name=f"att{hg}_{ct}")
                    nc.scalar.activation(
                        att[:].rearrange("p a b c -> p (a b c)"), flat, AF.Exp)
                    for cc in range(2):
                        c = 2 * ct + cc
                        for h in range(4):
                            for it in range(2):
                                s = h * 2 + it
                                # start=True clears has_written for the WHOLE
                                # bank; only the first slot of each 3-slot bank
                                # may set it (others overwrite-where-clear).
                                nc.tensor.matmul(
                                    ctx_slot(h, it),
                                    att[:, h, cc, it * 128:(it + 1) * 128],
                                    n1[:, c, :],
                                    start=(ct == 0 and cc == 0 and s % 3 == 0),
                                    stop=(ct == 7 and cc == 1))

                # epilogue: normalize ctx, transpose, apply value weights
                ctxn_sb = []
                for h in range(4):
                    for it in range(2):
                        rec = evacp.tile([D, 1], f32, tag="rec",
                                         name=f"rec{hg}_{h}_{it}")
                        nc.vector.reciprocal(rec[:], ctx_slot(h, it)[:, 128:129])
                        cn = evacp.tile([D, D], bfl, tag="ctxn",
                                        name=f"cn{hg}_{h}_{it}")
                        nc.vector.tensor_scalar_mul(
                            cn[:], ctx_slot(h, it)[:, 0:128], rec[:])
                        ctxn_sb.append(cn)
                for h in range(4):
                    for it in range(2):
                        cn = ctxn_sb[h * 2 + it]
                        tp = ctxp.tile([D, D], bfl, tag="ctx",
                                       name=f"tp{hg}_{h}_{it}")
                        nc.tensor.transpose(tp[:], cn[:], ident[:])
                        cnT = evacp.tile([D, D], bfl, tag="ctxnT",
                                         name=f"cnT{hg}_{h}_{it}")
                        nc.vector.tensor_copy(cnT[:], tp[:])
                        gh = hg * 4 + h
                        nc.tensor.matmul(
                            out_ps[:, it, :], cnT[:],
                            wv[:, gh * D:(gh + 1) * D],
                            start=(hg == 0 and h == 0 and it == 0),
                            stop=(hg == NHG - 1 and h == 3 and it == 1))

            # ---- finalize --------------------------------------------
            for it in range(2):
                fin = sb.tile([D, D], f32, name=f"fin{it}")
                nc.vector.tensor_add(fin[:], out_ps[:, it, :], bvr[:])
                nc.sync.dma_start(
                    out=outd[it * 128:(it + 1) * 128, :], in_=fin[:])

    _fix_sync_waits(nc)
    return nc


def _host_prep(nodes, pos, rot, w_nodes_kq, b_nodes_kq, w_pos_kq, b_pos_kq,
               w_rot_kq, w_values, b_values):
    """Pack weights/layouts; compute per-row offsets for the squared heads."""
    f32 = np.float32
    nodes = np.asarray(nodes, f32)
    pos = np.asarray(pos, f32)
    rot = np.asarray(rot, f32)

    pf = np.concatenate([np.cos(2 * np.pi * pos), np.sin(2 * np.pi * pos)],
                        axis=-1).astype(f32)
    aux = np.concatenate([pf, rot, np.ones((S, 1), f32)], axis=-1)  # [S, 11]

    nodesT = np.ascontiguousarray(nodes.T)                # [128, S]
    auxT = np.ascontiguousarray(aux.T)                    # [11, S]

    ones = np.ones((S, 1), f32)
    n1 = np.concatenate([nodes, ones], axis=1)            # [S, 129]
    n1 = n1.reshape(NCHUNK, 128, 129).transpose(1, 0, 2)  # [128, 16, 129]
    n1 = np.ascontiguousarray(n1.reshape(128, NCHUNK * 129)).astype(bf16)

    # nodes-family weight perm: [k h0-3 | k h4-7 | q h0-3 | q h4-7]
    k_rows = [h * 64 + c for h in range(8) for c in range(32)]
    q_rows = [h * 64 + 32 + c for h in range(8) for c in range(32)]
    perm = k_rows + q_rows
    WnT = np.ascontiguousarray(np.asarray(w_nodes_kq, f32)[perm, :].T)  # [128, 512]
    bn_perm = np.asarray(b_nodes_kq, f32)[perm]           # [512]
    bnT = np.ascontiguousarray(bn_perm.reshape(4, 128).T)  # [128, 4]

    # aux-family: [k_pos | k_rot | q_pos | q_rot] in hg order (hg2..hg5)
    Waux = np.zeros((1024, 11), f32)
    wp = np.asarray(w_pos_kq, f32)
    bp = np.asarray(b_pos_kq, f32)
    wr = np.asarray(w_rot_kq, f32)
    kp = [h * 64 + c for h in range(8) for c in range(32)]
    qp = [h * 64 + 32 + c for h in range(8) for c in range(32)]
    Waux[0:256, 0:6] = wp[kp, :]
    Waux[0:256, 10] = bp[kp]
    Waux[256:512, 6:10] = wr[kp, :]
    Waux[512:768, 0:6] = wp[qp, :]
    Waux[512:768, 10] = bp[qp]
    Waux[768:1024, 6:10] = wr[qp, :]
    WauxT = np.ascontiguousarray(Waux.T)                  # [11, 1024]

    WvT = np.ascontiguousarray(np.asarray(w_values, f32).T).astype(bf16)  # [128, 3072]
    bvs = np.asarray(b_values, f32).reshape(H3, D).sum(0).reshape(1, D)

    # per-row maxima of squared rot logits (host; cancel exactly in softmax)
    rkq = (rot @ wr.T).reshape(S, 8, 64)
    kr = rkq[..., :32]
    qr = rkq[..., 32:]
    Mrow = np.empty((8, S), f32)
    for h in range(8):
        lg = kr[:, h, :] @ qr[:, h, :].T                  # [i, j]
        Mrow[h] = (lg.astype(f32) ** 2).max(axis=1)

    return dict(nodesT=nodesT, auxT=auxT, n1=n1, WnT=WnT, bnT=bnT,
                WauxT=WauxT, WvT=WvT, bvs=bvs, Mrow=Mrow)


def kernel(**inputs):
    os.environ.setdefault("JAX_PLATFORMS", "axon")
    _install_env_patches()
    from concourse.bass_utils import run_bass_kernel_spmd

    hp = _host_prep(**inputs)

    if "nc" not in _PROGRAM_CACHE:
        _PROGRAM_CACHE["nc"] = build_program()
    nc = _PROGRAM_CACHE["nc"]

    shared = {k: hp[k] for k in
              ("nodesT", "auxT", "n1", "WnT", "bnT", "WauxT", "WvT", "bvs")}
    in_maps = []
    for core in range(NCORES):
        i0 = core * SL
        crep = np.empty((2, 2048), np.float32)
        for g in range(2):
            blk = hp["Mrow"][g * 4:(g + 1) * 4, i0:i0 + SL]   # [4, 256]
            crep[g] = np.repeat(-blk[:, None, :], 2, axis=1).reshape(2048)
        m = dict(shared)
        m["nodesT_loc"] = np.ascontiguousarray(hp["nodesT"][:, i0:i0 + SL])
        m["auxT_loc"] = np.ascontiguousarray(hp["auxT"][:, i0:i0 + SL])
        m["Crep"] = crep
        in_maps.append(m)

    res = run_bass_kernel_spmd(nc, in_maps, list(range(NCORES)),
                               trace=bool(int(os.environ.get("KTRACE", "0"))),
                               tmpdir=os.environ.get("KTRACE_DIR"))
    kernel.last_results = res
    out = np.concatenate([res.results[c]["out"] for c in range(NCORES)], axis=0)
    return out
